# revision 1
# baseline (speedup 1.0000x reference)
"""Trainium2 Bass kernel for nn_BiClassifier (bilinear pairwise MLP).

Math (per batch b):
    in1 = input1 @ W1.T + b1            # [N1, HID]
    in2 = input2 @ W2.T                 # [N2, HID]
    h   = relu(in1[:,None,:] + in2[None,:,:])   # [N1, N2, HID]  (never materialized)
    out = h @ Wo.T + bo                 # [N1, N2, OUT]

Strategy: shard the 512 (b, n1) rows across 8 cores (64 rows each, one batch
per core pair). Weights are replicated. On each core:
  Phase A (PE): produce in1T [HID, 64] and in2T [HID, 128] with HID on the
      partition axis (8 blocks of 128), using host-pre-transposed weights/inputs.
  Phase B: per row n, per hid-block hp, one tensor_scalar instruction computes
      relu(in2T_hp + in1T_hp[:, n]) -> h tile [128, 128]; the PE contracts
      h tiles [128, 512] (4 rows) against Wo embedded into [128, 16] stationary
      tiles (8 row-group slots x 2 outputs across 16 PSUM partitions), so one
      PSUM bank [16, 512] accumulates 32 rows of output.
Host pre/post: transpose/shard inputs, unscramble output, add bo.
"""

import sys

import numpy as np

_REPO = "/opt/trn_rl_repo"
if _REPO not in sys.path:
    sys.path.insert(0, _REPO)

import concourse.bass as bass
import concourse.mybir as mybir
import concourse.tile as tile
from concourse import bacc
from concourse.bass_utils import run_bass_kernel_spmd

B, N1, N2, D, HID, OUT = 4, 128, 128, 768, 1024, 2
NCORES = 8
NR = 64            # (b, n1) rows per core
DB = D // 128      # 6 contraction blocks for the input projections
HP = HID // 128    # 8 hid blocks
NSUP = 2           # row supers per core (32 rows each -> one PSUM bank)
NG = 8             # row groups per super
GR = 4             # rows per group (group -> one [128, 512] h tile)

# h-generation scheme:
#   "ts":  one fused tensor_scalar relu-add per row per hid block (FD=128).
#   "tt2": one broadcast tensor_tensor add per 4-row group (FD=512) plus an
#          in-place immediate-scalar relu pass (FD=512).
MODE = "ts"
# Engine weights (V=Vector, A=Scalar/ACT, G=GpSimd) for the add/relu work.
# Measured optimum (HW sweep): 93:35 Vector:Scalar — balances V's ~0.8us and
# ACT's ~1.6us effective per-group cost while keeping ACT below the PSUM-chain
# pacing threshold. GpSimd compute poisons DVE via the shared SBUF port; 0.
ADD_W = (93, 35, 0)
RELU_W = (1, 0, 0)
# Data dtype for weights/inputs/h tiles ("float32" or "bfloat16"). PSUM
# accumulation and the output stay fp32 either way.
DT = "bfloat16"

_CACHE = {}


def _wrr(weights, n):
    """Weighted round-robin schedule of 'V'/'A'/'G' over n slots."""
    names = "VAG"
    credits = [0.0, 0.0, 0.0]
    total = float(sum(weights))
    out = []
    for _ in range(n):
        credits = [c + w for c, w in zip(credits, weights)]
        i = max(range(3), key=lambda k: credits[k])
        credits[i] -= total
        out.append(names[i])
    return out


def _build(dt_name=None, mode=None, add_w=None, relu_w=None):
    f32 = mybir.dt.float32
    dt = getattr(mybir.dt, dt_name or DT)
    mode = mode or MODE
    n_groups = NSUP * HP * NG
    add_pat = _wrr(add_w or ADD_W, n_groups)
    relu_pat = _wrr(relu_w or RELU_W, n_groups)
    # Bacc (not plain Bass): its finalize() runs the walrus legalization
    # passes (move_matmul_waits_to_ldweights, event semaphores, ...) without
    # which multi-wait instructions fail neuronxcc codegen.
    nc = bacc.Bacc(None, target_bir_lowering=False)

    w1 = nc.declare_dram_parameter("w1", [128, HP * DB * 128], dt, isOutput=False)
    w2 = nc.declare_dram_parameter("w2", [128, HP * DB * 128], dt, isOutput=False)
    wote = nc.declare_dram_parameter("wote", [128, HP * NG * 16], dt, isOutput=False)
    b1s = nc.declare_dram_parameter("b1s", [128, HP], f32, isOutput=False)
    x1 = nc.declare_dram_parameter("x1", [128, DB * NR], dt, isOutput=False)
    x2 = nc.declare_dram_parameter("x2", [128, DB * N2], dt, isOutput=False)
    out = nc.declare_dram_parameter("out", [16, NSUP * 512], f32, isOutput=True)

    relu_idx = 0

    with tile.TileContext(nc) as tc:
        with (
            tc.tile_pool(name="const", bufs=1) as cpool,
            tc.tile_pool(name="wpool", bufs=1) as wpool,
            tc.tile_pool(name="hpool", bufs=12) as hpool,
            tc.tile_pool(name="pa", bufs=2, space=bass.MemorySpace.PSUM) as papool,
            tc.tile_pool(name="po", bufs=2, space=bass.MemorySpace.PSUM) as popool,
            tc.tile_pool(name="p2", bufs=1, space=bass.MemorySpace.PSUM) as p2pool,
        ):
            x1sb = cpool.tile([128, DB * NR], dt)
            x2sb = cpool.tile([128, DB * N2], dt)
            b1sb = cpool.tile([128, HP], f32)
            wotesb = cpool.tile([128, HP * NG * 16], dt)
            # in1t (fp32) feeds per-partition scalar reads (ACTIVATE bias /
            # tensor_scalar); in1tb (dt) feeds broadcast tensor_tensor adds.
            in1t = cpool.tile([128, HP * NR], f32)
            in1tb = (
                cpool.tile([128, HP * NR], dt, name="in1tb") if mode == "tt2" else None
            )
            in2t = cpool.tile([128, HP * N2], dt)
            outsb = cpool.tile([16, NSUP * 512], f32)

            # DMA order matters for pipeline fill: the first phase-A matmuls
            # need x1 + w1[0] (and x2 + w2[0]); defer b1/wote (phase-B-only).
            nc.sync.dma_start(out=x1sb[:], in_=x1[:])

            # Per-hp weight tiles so phase A hp can start as soon as its
            # slice lands (whole-W DMA would serialize ~20us at the front).
            w1sb = []
            w2sb = []
            for hp in range(HP):
                t1 = wpool.tile([128, DB * 128], dt, tag=f"w1_{hp}")
                t2 = wpool.tile([128, DB * 128], dt, tag=f"w2_{hp}")
                w1sb.append(t1)
                w2sb.append(t2)

            def _load_w(hp):
                nc.sync.dma_start(
                    out=w1sb[hp][:], in_=w1[:, hp * DB * 128 : (hp + 1) * DB * 128]
                )
                nc.sync.dma_start(
                    out=w2sb[hp][:], in_=w2[:, hp * DB * 128 : (hp + 1) * DB * 128]
                )

            _load_w(0)
            nc.sync.dma_start(out=x2sb[:], in_=x2[:])
            nc.sync.dma_start(out=b1sb[:], in_=b1s[:])
            _load_w(1)
            nc.sync.dma_start(out=wotesb[:], in_=wote[:])
            for hp in range(2, HP):
                _load_w(hp)

            # Dummy activation up front: pulls the ~2.7us ACT table load into
            # the DMA fill window instead of the first real relu.
            warm = cpool.tile([128, 1], f32, name="warm")
            nc.vector.memset(warm[:], 0.0)
            nc.scalar.activation(
                warm[:], warm[:], mybir.ActivationFunctionType.Relu, bias=0.0,
                scale=1.0,
            )

            # in2 projections stay resident in PSUM (2 banks, 4 hid blocks
            # each): ScalarE reads PSUM faster than SBUF, so 'A' groups
            # consume these directly; 'V' groups use the bf16 SBUF copy.
            in2ps = [
                p2pool.tile([128, 4 * N2], f32, name=f"in2ps{i}") for i in range(2)
            ]

            # ---- Phase A: in1T / in2T projections (hid on partitions) ----
            for hp in range(HP):
                ps1 = papool.tile([128, NR], f32, tag="ps1")
                for db in range(DB):
                    nc.tensor.matmul(
                        ps1[:],
                        w1sb[hp][:, db * 128 : (db + 1) * 128],
                        x1sb[:, db * NR : (db + 1) * NR],
                        start=(db == 0),
                        stop=(db == DB - 1),
                    )
                # in1t must stay fp32: it feeds tensor_scalar/activation
                # scalar reads, which are fp32-only.
                nc.vector.tensor_scalar_add(
                    in1t[:, hp * NR : (hp + 1) * NR], ps1[:], b1sb[:, hp : hp + 1]
                )
                if in1tb is not None:
                    nc.vector.tensor_copy(
                        in1tb[:, hp * NR : (hp + 1) * NR],
                        in1t[:, hp * NR : (hp + 1) * NR],
                    )

                ps2 = in2ps[hp // 4][:, (hp % 4) * N2 : (hp % 4 + 1) * N2]
                for db in range(DB):
                    nc.tensor.matmul(
                        ps2,
                        w2sb[hp][:, db * 128 : (db + 1) * 128],
                        x2sb[:, db * N2 : (db + 1) * N2],
                        start=(db == 0),
                        stop=(db == DB - 1),
                    )
                nc.scalar.copy(in2t[:, hp * N2 : (hp + 1) * N2], ps2)

            # ---- Phase B: relu outer-sum + projection onto Wo ----
            for sup in range(NSUP):
                pso = popool.tile([16, 512], f32, tag="pso")
                for hp in range(HP):
                    for g in range(NG):
                        h = hpool.tile([128, GR * N2], dt, tag="h")
                        # One engine per pass per h tile: mixed producers
                        # would pile sync-waits onto the consuming matmul
                        # (walrus caps wait commands per instruction).
                        which = add_pat[relu_idx]
                        relu_idx += 1
                        r0 = sup * (NG * GR) + g * GR
                        src = in2t[:, hp * N2 : (hp + 1) * N2]
                        if mode == "tt2" and which != "A":
                            # broadcast TT add (V or G) + in-place relu (V)
                            eng = nc.vector if which == "V" else nc.gpsimd
                            a = src[:, None, :].broadcast_to([128, GR, N2])
                            b = in1tb[:, hp * NR + r0 : hp * NR + r0 + GR][
                                :, :, None
                            ].broadcast_to([128, GR, N2])
                            hv = h[:].rearrange("p (n m) -> p n m", n=GR)
                            eng.tensor_tensor(hv, a, b, mybir.AluOpType.add)
                            nc.vector.tensor_scalar(
                                h[:], h[:], 0.0, None, mybir.AluOpType.max
                            )
                        else:
                            psrc = in2ps[hp // 4][:, (hp % 4) * N2 : (hp % 4 + 1) * N2]
                            for j in range(GR):
                                row = r0 + j
                                col = in1t[:, hp * NR + row : hp * NR + row + 1]
                                dst = h[:, j * N2 : (j + 1) * N2]
                                if which == "A":
                                    nc.scalar.activation(
                                        dst,
                                        psrc,
                                        mybir.ActivationFunctionType.Relu,
                                        bias=col,
                                        scale=1.0,
                                    )
                                else:
                                    nc.vector.tensor_scalar(
                                        dst,
                                        src,
                                        col,
                                        0.0,
                                        mybir.AluOpType.add,
                                        mybir.AluOpType.max,
                                    )
                        nc.tensor.matmul(
                            pso[:],
                            wotesb[:, hp * NG * 16 + g * 16 : hp * NG * 16 + (g + 1) * 16],
                            h[:],
                            start=(hp == 0 and g == 0),
                            stop=(hp == HP - 1 and g == NG - 1),
                        )
                nc.vector.tensor_copy(outsb[:, sup * 512 : (sup + 1) * 512], pso[:])
                nc.sync.dma_start(
                    out=out[:, sup * 512 : (sup + 1) * 512],
                    in_=outsb[:, sup * 512 : (sup + 1) * 512],
                )

    nc.finalize()
    return nc


def _np_dt(dt_name):
    if dt_name == "bfloat16":
        import ml_dtypes

        return ml_dtypes.bfloat16
    return np.float32


def _host_prep(input1, input2, W1, b1, W2, Wo, dt_name=None):
    f32 = np.float32
    dt = _np_dt(dt_name or DT)
    c = np.ascontiguousarray

    # w[p, hp, db, j] = W[hp*128+j, db*128+p]
    w1sb = c(W1.reshape(HP, 128, DB, 128).transpose(3, 0, 2, 1).reshape(128, -1), dt)
    w2sb = c(W2.reshape(HP, 128, DB, 128).transpose(3, 0, 2, 1).reshape(128, -1), dt)

    # wote[p, hp, s, 2s+o] = Wo[o, hp*128+p]
    wo_hpo = Wo.T.reshape(HP, 128, OUT)  # [hp, p, o]
    wote = np.zeros((128, HP, NG, 16), f32)
    for s in range(NG):
        wote[:, :, s, 2 * s : 2 * s + 2] = wo_hpo.transpose(1, 0, 2)
    wote = c(wote.reshape(128, -1), dt)

    b1sb = c(b1.reshape(HP, 128).T, f32)

    in_maps = []
    for core in range(NCORES):
        b, half = core // 2, core % 2
        n0 = half * NR
        x1sb = c(
            input1[b, n0 : n0 + NR].reshape(NR, DB, 128).transpose(2, 1, 0).reshape(128, -1),
            dt,
        )
        x2sb = c(
            input2[b].reshape(N2, DB, 128).transpose(2, 1, 0).reshape(128, -1), dt
        )
        in_maps.append(
            {"w1": w1sb, "w2": w2sb, "wote": wote, "b1s": b1sb, "x1": x1sb, "x2": x2sb}
        )
    return in_maps


def _host_post(results, bo):
    out_full = np.empty((B, N1, N2, OUT), np.float32)
    for core in range(NCORES):
        b, half = core // 2, core % 2
        co = np.asarray(results[core]["out"], np.float32)
        co = co.reshape(NG, OUT, NSUP, GR, N2)  # [s, o, sup, j, m]
        arr = co.transpose(2, 0, 3, 4, 1).reshape(NR, N2, OUT)  # [sup,s,j] -> rows
        out_full[b, half * NR : (half + 1) * NR] = arr
    out_full += np.asarray(bo, np.float32)
    return out_full


def run(inputs, trace=False, dt_name=None, mode=None, add_w=None, relu_w=None,
        **spmd_kwargs):
    """Run on hardware; returns (output, BassKernelResults)."""
    key = (dt_name or DT, mode or MODE, add_w or ADD_W, relu_w or RELU_W)
    if key not in _CACHE:
        _CACHE[key] = _build(dt_name=dt_name, mode=mode, add_w=add_w, relu_w=relu_w)
    nc = _CACHE[key]
    in_maps = _host_prep(
        np.asarray(inputs["input1"], np.float32),
        np.asarray(inputs["input2"], np.float32),
        np.asarray(inputs["W1"], np.float32),
        np.asarray(inputs["b1"], np.float32),
        np.asarray(inputs["W2"], np.float32),
        np.asarray(inputs["Wo"], np.float32),
        dt_name=dt_name,
    )
    res = run_bass_kernel_spmd(
        nc, in_maps, list(range(NCORES)), trace=trace, **spmd_kwargs
    )
    out = _host_post(res.results, np.asarray(inputs["bo"], np.float32))
    return out, res


def kernel(**inputs) -> np.ndarray:
    out, _ = run(inputs, trace=False)
    return out


if __name__ == "__main__":
    rng = np.random.default_rng(0)
    ins = {
        "input1": rng.standard_normal((B, N1, D), dtype=np.float32),
        "input2": rng.standard_normal((B, N2, D), dtype=np.float32),
        "W1": rng.standard_normal((HID, D), dtype=np.float32) * 0.036,
        "b1": rng.standard_normal((HID,), dtype=np.float32) * 0.036,
        "W2": rng.standard_normal((HID, D), dtype=np.float32) * 0.036,
        "Wo": rng.standard_normal((OUT, HID), dtype=np.float32) * 0.031,
        "bo": rng.standard_normal((OUT,), dtype=np.float32) * 0.031,
    }
    out = kernel(**ins)
    print("kernel out", out.shape, out.dtype)



# revision 3
# speedup vs baseline: 1.0122x; 1.0122x over previous
"""Trainium2 Bass kernel for nn_BiClassifier (bilinear pairwise MLP).

Math (per batch b):
    in1 = input1 @ W1.T + b1            # [N1, HID]
    in2 = input2 @ W2.T                 # [N2, HID]
    h   = relu(in1[:,None,:] + in2[None,:,:])   # [N1, N2, HID]  (never materialized)
    out = h @ Wo.T + bo                 # [N1, N2, OUT]

Strategy: shard the 512 (b, n1) rows across 8 cores (64 rows each, one batch
per core pair). Weights are replicated. On each core the work is streamed
per hid-block hp (8 blocks of 128 on the partition axis):
  Phase A(hp) on PE: in1T[:, hp] [128, 64] and in2T[:, hp] [128, 128] from
      host-pre-transposed weights/inputs; in2T stays resident in PSUM for the
      ACT consumers and is copied to SBUF bf16 for the DVE consumers.
  Phase B(hp): 8 double-tiles h2 [128, 1024] = relu(in2T + in1T[:, n]) for
      group g's 4 rows of BOTH row-supers (one tensor_scalar / activation per
      row, FD=128 — per-partition scalar caps FD). Each h2 feeds two PE
      matmuls against the same wote [128, 16] stationary (Wo embedded at psum
      rows 2g:2g+2), accumulating [16, 512] PSUM banks (one per super) across
      all hp. Phase A(hp+1) is emitted after g0 so PE prefetches it into the
      producer-limited window.
Producers are split V:A by weighted round-robin (~71:29 balances DVE's
~165ns/instr against ACT's ~400ns/instr; both are dispatch-bound).
Host pre/post: transpose/shard inputs, unscramble output, add bo.
"""

import sys

import numpy as np

_REPO = "/opt/trn_rl_repo"
if _REPO not in sys.path:
    sys.path.insert(0, _REPO)

import concourse.bass as bass
import concourse.mybir as mybir
import concourse.tile as tile
from concourse import bacc
from concourse.bass_utils import run_bass_kernel_spmd

B, N1, N2, D, HID, OUT = 4, 128, 128, 768, 1024, 2
NCORES = 8
NR = 64            # (b, n1) rows per core
DB = D // 128      # 6 contraction blocks for the input projections
HP = HID // 128    # 8 hid blocks
NSUP = 2           # row supers per core (32 rows each -> one PSUM bank)
NG = 8             # row groups per super
GR = 4             # rows per group (group -> one [128, 512] h half-tile)

# Engine weights (V=Vector, A=Scalar/ACT) for h2-tile production.
ADD_W = (71, 29)
# Data dtype for weights/inputs/h tiles. PSUM accumulation and out stay fp32.
DT = "bfloat16"

_CACHE = {}


def _wrr(weights, n):
    """Weighted round-robin schedule of 'V'/'A' over n slots."""
    names = "VA"
    credits = [0.0, 0.0]
    total = float(sum(weights))
    out = []
    for _ in range(n):
        credits = [c + w for c, w in zip(credits, weights)]
        i = max(range(2), key=lambda k: credits[k])
        credits[i] -= total
        out.append(names[i])
    return out


def _build(dt_name=None, add_w=None):
    f32 = mybir.dt.float32
    dt = getattr(mybir.dt, dt_name or DT)
    sched = _wrr(add_w or ADD_W, HP * NG)
    # Bacc (not plain Bass): its finalize() runs the walrus legalization
    # passes (move_matmul_waits_to_ldweights, event semaphores, ...) without
    # which multi-wait instructions fail neuronxcc codegen.
    nc = bacc.Bacc(None, target_bir_lowering=False)

    w1 = nc.declare_dram_parameter("w1", [128, HP * DB * 128], dt, isOutput=False)
    w2 = nc.declare_dram_parameter("w2", [128, HP * DB * 128], dt, isOutput=False)
    wote = nc.declare_dram_parameter("wote", [128, HP * NG * 16], dt, isOutput=False)
    b1s = nc.declare_dram_parameter("b1s", [128, HP], f32, isOutput=False)
    x1 = nc.declare_dram_parameter("x1", [128, DB * NR], dt, isOutput=False)
    x2 = nc.declare_dram_parameter("x2", [128, DB * N2], dt, isOutput=False)
    out = nc.declare_dram_parameter("out", [16, NSUP * 512], f32, isOutput=True)

    with tile.TileContext(nc) as tc:
        with (
            tc.tile_pool(name="const", bufs=1) as cpool,
            tc.tile_pool(name="wpool", bufs=1) as wpool,
            tc.tile_pool(name="hpool", bufs=6) as hpool,
            tc.tile_pool(name="pa1", bufs=2, space=bass.MemorySpace.PSUM) as pa1,
            tc.tile_pool(name="pa2", bufs=2, space=bass.MemorySpace.PSUM) as pa2,
            tc.tile_pool(name="po", bufs=2, space=bass.MemorySpace.PSUM) as po,
        ):
            x1sb = cpool.tile([128, DB * NR], dt)
            x2sb = cpool.tile([128, DB * N2], dt)
            b1sb = cpool.tile([128, HP], f32)
            wotesb = cpool.tile([128, HP * NG * 16], dt)
            # in1t (fp32) feeds per-partition scalar reads (ACTIVATE bias /
            # tensor_scalar), which are fp32-only.
            in1t = cpool.tile([128, HP * NR], f32)
            in2t = cpool.tile([128, HP * N2], dt)
            outsb = cpool.tile([16, NSUP * 512], f32)

            # Per-hp weight tiles so phase A hp can start as soon as its
            # slice lands.
            w1sb = []
            w2sb = []
            for hp in range(HP):
                t1 = wpool.tile([128, DB * 128], dt, tag=f"w1_{hp}")
                t2 = wpool.tile([128, DB * 128], dt, tag=f"w2_{hp}")
                w1sb.append(t1)
                w2sb.append(t2)

            def _load_w(hp):
                nc.sync.dma_start(
                    out=w1sb[hp][:], in_=w1[:, hp * DB * 128 : (hp + 1) * DB * 128]
                )
                nc.sync.dma_start(
                    out=w2sb[hp][:], in_=w2[:, hp * DB * 128 : (hp + 1) * DB * 128]
                )

            # DMA order = pipeline fill order: first phase-A needs x1/x2 +
            # w[0]; b1/wote are phase-B-only and can trail w[1].
            nc.sync.dma_start(out=x1sb[:], in_=x1[:])
            _load_w(0)
            nc.sync.dma_start(out=x2sb[:], in_=x2[:])
            nc.sync.dma_start(out=b1sb[:], in_=b1s[:])
            _load_w(1)
            nc.sync.dma_start(out=wotesb[:], in_=wote[:])
            for hp in range(2, HP):
                _load_w(hp)

            # Dummy activation up front: pulls the ~2.7us ACT table load into
            # the DMA fill window instead of the first real relu.
            warm = cpool.tile([128, 1], f32, name="warm")
            nc.vector.memset(warm[:], 0.0)
            nc.scalar.activation(
                warm[:], warm[:], mybir.ActivationFunctionType.Relu, bias=0.0,
                scale=1.0,
            )

            # Output accumulators: one [16, 512] bank per row-super, live for
            # the whole phase-B accumulation (all hp).
            pso = [po.tile([16, 512], f32, name=f"pso{s}") for s in range(NSUP)]

            def phase_a(hp):
                ps1 = pa1.tile([128, NR], f32, tag="ps1")
                for db in range(DB):
                    nc.tensor.matmul(
                        ps1[:],
                        w1sb[hp][:, db * 128 : (db + 1) * 128],
                        x1sb[:, db * NR : (db + 1) * NR],
                        start=(db == 0),
                        stop=(db == DB - 1),
                    )
                ps2 = pa2.tile([128, N2], f32, tag="ps2")
                for db in range(DB):
                    nc.tensor.matmul(
                        ps2[:],
                        w2sb[hp][:, db * 128 : (db + 1) * 128],
                        x2sb[:, db * N2 : (db + 1) * N2],
                        start=(db == 0),
                        stop=(db == DB - 1),
                    )
                return ps1, ps2

            def v_aux(hp, ps1, ps2):
                # in1t slice (fp32, += b1) and bf16 SBUF copy of in2T.
                nc.vector.tensor_scalar_add(
                    in1t[:, hp * NR : (hp + 1) * NR], ps1[:], b1sb[:, hp : hp + 1]
                )
                nc.vector.tensor_copy(in2t[:, hp * N2 : (hp + 1) * N2], ps2[:])

            cur = phase_a(0)
            v_aux(0, *cur)

            for hp in range(HP):
                ps2 = cur[1]
                src = in2t[:, hp * N2 : (hp + 1) * N2]
                for g in range(NG):
                    h2 = hpool.tile([128, NSUP * GR * N2], dt, tag="h")
                    which = sched[hp * NG + g]
                    for sup in range(NSUP):
                        for j in range(GR):
                            row = sup * (NG * GR) + g * GR + j
                            col = in1t[:, hp * NR + row : hp * NR + row + 1]
                            dst = h2[:, (sup * GR + j) * N2 : (sup * GR + j + 1) * N2]
                            if which == "A":
                                nc.scalar.activation(
                                    dst,
                                    ps2[:],
                                    mybir.ActivationFunctionType.Relu,
                                    bias=col,
                                    scale=1.0,
                                )
                            else:
                                nc.vector.tensor_scalar(
                                    dst,
                                    src,
                                    col,
                                    0.0,
                                    mybir.AluOpType.add,
                                    mybir.AluOpType.max,
                                )
                    wslice = wotesb[
                        :, hp * NG * 16 + g * 16 : hp * NG * 16 + (g + 1) * 16
                    ]
                    for sup in range(NSUP):
                        nc.tensor.matmul(
                            pso[sup][:],
                            wslice,
                            h2[:, sup * GR * N2 : (sup + 1) * GR * N2],
                            start=(hp == 0 and g == 0),
                            stop=(hp == HP - 1 and g == NG - 1),
                        )
                    if g == 0 and hp + 1 < HP:
                        # Prefetch next hp's projections into the PE bubble;
                        # producers for hp still have ~7 tiles queued.
                        cur = phase_a(hp + 1)
                        v_aux(hp + 1, *cur)

            for sup in range(NSUP):
                nc.vector.tensor_copy(
                    outsb[:, sup * 512 : (sup + 1) * 512], pso[sup][:]
                )
                nc.sync.dma_start(
                    out=out[:, sup * 512 : (sup + 1) * 512],
                    in_=outsb[:, sup * 512 : (sup + 1) * 512],
                )

    nc.finalize()
    return nc


def _np_dt(dt_name):
    if dt_name == "bfloat16":
        import ml_dtypes

        return ml_dtypes.bfloat16
    return np.float32


def _host_prep(input1, input2, W1, b1, W2, Wo, dt_name=None):
    f32 = np.float32
    dt = _np_dt(dt_name or DT)
    c = np.ascontiguousarray

    # w[p, hp, db, j] = W[hp*128+j, db*128+p]
    w1sb = c(W1.reshape(HP, 128, DB, 128).transpose(3, 0, 2, 1).reshape(128, -1), dt)
    w2sb = c(W2.reshape(HP, 128, DB, 128).transpose(3, 0, 2, 1).reshape(128, -1), dt)

    # wote[p, hp, s, 2s+o] = Wo[o, hp*128+p]
    wo_hpo = Wo.T.reshape(HP, 128, OUT)  # [hp, p, o]
    wote = np.zeros((128, HP, NG, 16), f32)
    for s in range(NG):
        wote[:, :, s, 2 * s : 2 * s + 2] = wo_hpo.transpose(1, 0, 2)
    wote = c(wote.reshape(128, -1), dt)

    b1sb = c(b1.reshape(HP, 128).T, f32)

    in_maps = []
    for core in range(NCORES):
        b, half = core // 2, core % 2
        n0 = half * NR
        x1sb = c(
            input1[b, n0 : n0 + NR].reshape(NR, DB, 128).transpose(2, 1, 0).reshape(128, -1),
            dt,
        )
        x2sb = c(
            input2[b].reshape(N2, DB, 128).transpose(2, 1, 0).reshape(128, -1), dt
        )
        in_maps.append(
            {"w1": w1sb, "w2": w2sb, "wote": wote, "b1s": b1sb, "x1": x1sb, "x2": x2sb}
        )
    return in_maps


def _host_post(results, bo):
    out_full = np.empty((B, N1, N2, OUT), np.float32)
    for core in range(NCORES):
        b, half = core // 2, core % 2
        co = np.asarray(results[core]["out"], np.float32)
        co = co.reshape(NG, OUT, NSUP, GR, N2)  # [s, o, sup, j, m]
        arr = co.transpose(2, 0, 3, 4, 1).reshape(NR, N2, OUT)  # [sup,s,j] -> rows
        out_full[b, half * NR : (half + 1) * NR] = arr
    out_full += np.asarray(bo, np.float32)
    return out_full


def run(inputs, trace=False, dt_name=None, add_w=None, **spmd_kwargs):
    """Run on hardware; returns (output, BassKernelResults)."""
    key = (dt_name or DT, add_w or ADD_W)
    if key not in _CACHE:
        _CACHE[key] = _build(dt_name=dt_name, add_w=add_w)
    nc = _CACHE[key]
    in_maps = _host_prep(
        np.asarray(inputs["input1"], np.float32),
        np.asarray(inputs["input2"], np.float32),
        np.asarray(inputs["W1"], np.float32),
        np.asarray(inputs["b1"], np.float32),
        np.asarray(inputs["W2"], np.float32),
        np.asarray(inputs["Wo"], np.float32),
        dt_name=dt_name,
    )
    res = run_bass_kernel_spmd(
        nc, in_maps, list(range(NCORES)), trace=trace, **spmd_kwargs
    )
    out = _host_post(res.results, np.asarray(inputs["bo"], np.float32))
    return out, res


def kernel(**inputs) -> np.ndarray:
    out, _ = run(inputs, trace=False)
    return out


if __name__ == "__main__":
    rng = np.random.default_rng(0)
    ins = {
        "input1": rng.standard_normal((B, N1, D), dtype=np.float32),
        "input2": rng.standard_normal((B, N2, D), dtype=np.float32),
        "W1": rng.standard_normal((HID, D), dtype=np.float32) * 0.036,
        "b1": rng.standard_normal((HID,), dtype=np.float32) * 0.036,
        "W2": rng.standard_normal((HID, D), dtype=np.float32) * 0.036,
        "Wo": rng.standard_normal((OUT, HID), dtype=np.float32) * 0.031,
        "bo": rng.standard_normal((OUT,), dtype=np.float32) * 0.031,
    }
    out = kernel(**ins)
    print("kernel out", out.shape, out.dtype)


# revision 6
# speedup vs baseline: 1.0284x; 1.0160x over previous
"""Trainium2 Bass kernel for nn_BiClassifier (bilinear pairwise MLP).

Math (per batch b):
    in1 = input1 @ W1.T + b1            # [N1, HID]
    in2 = input2 @ W2.T                 # [N2, HID]
    h   = relu(in1[:,None,:] + in2[None,:,:])   # [N1, N2, HID]  (never materialized)
    out = h @ Wo.T + bo                 # [N1, N2, OUT]

Strategy: shard the 512 (b, n1) rows across 8 cores (64 rows each, one batch
per core pair). Weights are replicated. On each core the work is streamed
per hid-block hp (8 blocks of 128 on the partition axis):
  Phase A(hp) on PE: in1T[:, hp] [128, 64] and in2T[:, hp] [128, 128] from
      host-pre-transposed weights/inputs; in2T stays resident in PSUM for the
      ACT consumers and is copied to SBUF bf16 for the DVE consumers.
  Phase B(hp): 8 double-tiles h2 [128, 1024] = relu(in2T + in1T[:, n]) for
      group g's 4 rows of BOTH row-supers (one tensor_scalar / activation per
      row, FD=128 — per-partition scalar caps FD). Each h2 feeds two PE
      matmuls against the same wote [128, 16] stationary (Wo embedded at psum
      rows 2g:2g+2), accumulating [16, 512] PSUM banks (one per super) across
      all hp. Phase A(hp+1) is emitted after g0 so PE prefetches it into the
      producer-limited window.
Producers are split V:A by weighted round-robin (~71:29 balances DVE's
~165ns/instr against ACT's ~400ns/instr; both are dispatch-bound).
Host pre/post: transpose/shard inputs, unscramble output, add bo.
"""

import sys

import numpy as np

_REPO = "/opt/trn_rl_repo"
if _REPO not in sys.path:
    sys.path.insert(0, _REPO)

import concourse.bass as bass
import concourse.mybir as mybir
import concourse.tile as tile
from concourse import bacc
from concourse.bass_utils import run_bass_kernel_spmd

B, N1, N2, D, HID, OUT = 4, 128, 128, 768, 1024, 2
NCORES = 8
NR = 64            # (b, n1) rows per core
DB = D // 128      # 6 contraction blocks for the input projections
HP = HID // 128    # 8 hid blocks
NSUP = 2           # row supers per core (32 rows each -> one PSUM bank)
NG = 8             # row groups per super
GR = 4             # rows per group (group -> one [128, 512] h half-tile)

# Engine weights (V=Vector, A=Scalar/ACT) for h2-tile production.
ADD_W = (71, 29)
# Data dtype for weights/inputs/h tiles. PSUM accumulation and out stay fp32.
DT = "bfloat16"

_CACHE = {}


def _wrr(weights, n):
    """Weighted round-robin schedule of 'V'/'A' over n slots."""
    names = "VA"
    credits = [0.0, 0.0]
    total = float(sum(weights))
    out = []
    for _ in range(n):
        credits = [c + w for c, w in zip(credits, weights)]
        i = max(range(2), key=lambda k: credits[k])
        credits[i] -= total
        out.append(names[i])
    return out


def _build(dt_name=None, add_w=None):
    f32 = mybir.dt.float32
    dt = getattr(mybir.dt, dt_name or DT)
    sched = _wrr(add_w or ADD_W, HP * NG)
    # Bacc (not plain Bass): its finalize() runs the walrus legalization
    # passes (move_matmul_waits_to_ldweights, event semaphores, ...) without
    # which multi-wait instructions fail neuronxcc codegen.
    nc = bacc.Bacc(None, target_bir_lowering=False)

    w1 = nc.declare_dram_parameter("w1", [128, HP * DB * 128], dt, isOutput=False)
    w2 = nc.declare_dram_parameter("w2", [128, HP * DB * 128], dt, isOutput=False)
    wote = nc.declare_dram_parameter("wote", [128, HP * NG * 16], dt, isOutput=False)
    b1s = nc.declare_dram_parameter("b1s", [128, HP], f32, isOutput=False)
    x1 = nc.declare_dram_parameter("x1", [128, DB * NR], dt, isOutput=False)
    x2 = nc.declare_dram_parameter("x2", [128, DB * N2], dt, isOutput=False)
    out = nc.declare_dram_parameter("out", [16, NSUP * 512], f32, isOutput=True)

    with tile.TileContext(nc) as tc:
        with (
            tc.tile_pool(name="const", bufs=1) as cpool,
            tc.tile_pool(name="wpool", bufs=1) as wpool,
            tc.tile_pool(name="hpool", bufs=6) as hpool,
            tc.tile_pool(name="pa1", bufs=2, space=bass.MemorySpace.PSUM) as pa1,
            tc.tile_pool(name="pa2", bufs=2, space=bass.MemorySpace.PSUM) as pa2,
            tc.tile_pool(name="po", bufs=2, space=bass.MemorySpace.PSUM) as po,
        ):
            x1sb = cpool.tile([128, DB * NR], dt)
            x2sb = cpool.tile([128, DB * N2], dt)
            b1sb = cpool.tile([128, HP], f32)
            wotesb = cpool.tile([128, HP * NG * 16], dt)
            # in1t (fp32) feeds per-partition scalar reads (ACTIVATE bias /
            # tensor_scalar), which are fp32-only.
            in1t = cpool.tile([128, HP * NR], f32)
            in2t = cpool.tile([128, HP * N2], dt)
            outsb = cpool.tile([16, NSUP * 512], f32)

            # Per-hp weight tiles so phase A hp can start as soon as its
            # slice lands.
            w1sb = []
            w2sb = []
            for hp in range(HP):
                t1 = wpool.tile([128, DB * 128], dt, tag=f"w1_{hp}")
                t2 = wpool.tile([128, DB * 128], dt, tag=f"w2_{hp}")
                w1sb.append(t1)
                w2sb.append(t2)

            def _load_w(hp, eng=None):
                (eng or nc.sync).dma_start(
                    out=w1sb[hp][:], in_=w1[:, hp * DB * 128 : (hp + 1) * DB * 128]
                )
                (eng or nc.sync).dma_start(
                    out=w2sb[hp][:], in_=w2[:, hp * DB * 128 : (hp + 1) * DB * 128]
                )

            # DMA fill: the ramp is bounded by issue cost (~0.6us per
            # dma_start, serial per queue) plus ~3us transfer per 196KB tile,
            # so the hp0-critical tiles are split in half and issued in
            # parallel across the two HWDGE queues (sync + scalar).
            HB = DB // 2 * 128
            nc.sync.dma_start(out=w1sb[0][:, :HB], in_=w1[:, :HB])
            nc.scalar.dma_start(out=x1sb[:], in_=x1[:])
            nc.sync.dma_start(out=w1sb[0][:, HB:], in_=w1[:, HB : DB * 128])
            nc.scalar.dma_start(out=b1sb[:], in_=b1s[:])
            nc.sync.dma_start(out=x2sb[:], in_=x2[:])
            nc.scalar.dma_start(out=w2sb[0][:, :HB], in_=w2[:, :HB])
            nc.scalar.dma_start(
                out=w2sb[0][:, HB:], in_=w2[:, HB : DB * 128]
            )
            _load_w(1)
            nc.sync.dma_start(out=wotesb[:], in_=wote[:])
            for hp in range(2, HP):
                _load_w(hp)

            # Dummy activation up front: pulls the ~2.7us ACT table load into
            # the DMA fill window instead of the first real relu.
            warm = cpool.tile([128, 1], f32, name="warm")
            nc.vector.memset(warm[:], 0.0)
            nc.scalar.activation(
                warm[:], warm[:], mybir.ActivationFunctionType.Relu, bias=0.0,
                scale=1.0,
            )

            # Output accumulators: one [16, 512] bank per row-super, live for
            # the whole phase-B accumulation (all hp).
            pso = [po.tile([16, 512], f32, name=f"pso{s}") for s in range(NSUP)]

            def phase_a(hp):
                ps1 = pa1.tile([128, NR], f32, tag="ps1")
                for db in range(DB):
                    nc.tensor.matmul(
                        ps1[:],
                        w1sb[hp][:, db * 128 : (db + 1) * 128],
                        x1sb[:, db * NR : (db + 1) * NR],
                        start=(db == 0),
                        stop=(db == DB - 1),
                    )
                ps2 = pa2.tile([128, N2], f32, tag="ps2")
                for db in range(DB):
                    nc.tensor.matmul(
                        ps2[:],
                        w2sb[hp][:, db * 128 : (db + 1) * 128],
                        x2sb[:, db * N2 : (db + 1) * N2],
                        start=(db == 0),
                        stop=(db == DB - 1),
                    )
                return ps1, ps2

            def a_aux(hp, ps1, ps2):
                # in1t slice (fp32, += b1, Identity keeps negatives) and bf16
                # SBUF copy of in2T — both on ACT to keep the DVE queue clean.
                nc.scalar.activation(
                    in1t[:, hp * NR : (hp + 1) * NR],
                    ps1[:],
                    mybir.ActivationFunctionType.Identity,
                    bias=b1sb[:, hp : hp + 1],
                    scale=1.0,
                )
                nc.scalar.copy(in2t[:, hp * N2 : (hp + 1) * N2], ps2[:])

            cur = phase_a(0)
            a_aux(0, *cur)

            for hp in range(HP):
                ps2 = cur[1]
                src = in2t[:, hp * N2 : (hp + 1) * N2]
                for g in range(NG):
                    h2 = hpool.tile([128, NSUP * GR * N2], dt, tag="h")
                    which = sched[hp * NG + g]
                    for sup in range(NSUP):
                        for j in range(GR):
                            row = sup * (NG * GR) + g * GR + j
                            col = in1t[:, hp * NR + row : hp * NR + row + 1]
                            dst = h2[:, (sup * GR + j) * N2 : (sup * GR + j + 1) * N2]
                            if which == "A":
                                nc.scalar.activation(
                                    dst,
                                    ps2[:],
                                    mybir.ActivationFunctionType.Relu,
                                    bias=col,
                                    scale=1.0,
                                )
                            else:
                                nc.vector.tensor_scalar(
                                    dst,
                                    src,
                                    col,
                                    0.0,
                                    mybir.AluOpType.add,
                                    mybir.AluOpType.max,
                                )
                    wslice = wotesb[
                        :, hp * NG * 16 + g * 16 : hp * NG * 16 + (g + 1) * 16
                    ]
                    for sup in range(NSUP):
                        nc.tensor.matmul(
                            pso[sup][:],
                            wslice,
                            h2[:, sup * GR * N2 : (sup + 1) * GR * N2],
                            start=(hp == 0 and g == 0),
                            stop=(hp == HP - 1 and g == NG - 1),
                        )
                    if g == 0 and hp + 1 < HP:
                        # Prefetch next hp's projections into the PE bubble;
                        # producers for hp still have ~7 tiles queued.
                        cur = phase_a(hp + 1)
                        a_aux(hp + 1, *cur)

            # Evacuate on both engines in parallel to shorten the tail.
            nc.vector.tensor_copy(outsb[:, 0:512], pso[0][:])
            nc.scalar.copy(outsb[:, 512:1024], pso[1][:])
            nc.sync.dma_start(out=out[:], in_=outsb[:])

    nc.finalize()
    return nc


def _np_dt(dt_name):
    if dt_name == "bfloat16":
        import ml_dtypes

        return ml_dtypes.bfloat16
    return np.float32


def _host_prep(input1, input2, W1, b1, W2, Wo, dt_name=None):
    f32 = np.float32
    dt = _np_dt(dt_name or DT)
    c = np.ascontiguousarray

    # w[p, hp, db, j] = W[hp*128+j, db*128+p]
    w1sb = c(W1.reshape(HP, 128, DB, 128).transpose(3, 0, 2, 1).reshape(128, -1), dt)
    w2sb = c(W2.reshape(HP, 128, DB, 128).transpose(3, 0, 2, 1).reshape(128, -1), dt)

    # wote[p, hp, s, 2s+o] = Wo[o, hp*128+p]
    wo_hpo = Wo.T.reshape(HP, 128, OUT)  # [hp, p, o]
    wote = np.zeros((128, HP, NG, 16), f32)
    for s in range(NG):
        wote[:, :, s, 2 * s : 2 * s + 2] = wo_hpo.transpose(1, 0, 2)
    wote = c(wote.reshape(128, -1), dt)

    b1sb = c(b1.reshape(HP, 128).T, f32)

    in_maps = []
    for core in range(NCORES):
        b, half = core // 2, core % 2
        n0 = half * NR
        x1sb = c(
            input1[b, n0 : n0 + NR].reshape(NR, DB, 128).transpose(2, 1, 0).reshape(128, -1),
            dt,
        )
        x2sb = c(
            input2[b].reshape(N2, DB, 128).transpose(2, 1, 0).reshape(128, -1), dt
        )
        in_maps.append(
            {"w1": w1sb, "w2": w2sb, "wote": wote, "b1s": b1sb, "x1": x1sb, "x2": x2sb}
        )
    return in_maps


def _host_post(results, bo):
    out_full = np.empty((B, N1, N2, OUT), np.float32)
    for core in range(NCORES):
        b, half = core // 2, core % 2
        co = np.asarray(results[core]["out"], np.float32)
        co = co.reshape(NG, OUT, NSUP, GR, N2)  # [s, o, sup, j, m]
        arr = co.transpose(2, 0, 3, 4, 1).reshape(NR, N2, OUT)  # [sup,s,j] -> rows
        out_full[b, half * NR : (half + 1) * NR] = arr
    out_full += np.asarray(bo, np.float32)
    return out_full


def run(inputs, trace=False, dt_name=None, add_w=None, **spmd_kwargs):
    """Run on hardware; returns (output, BassKernelResults)."""
    key = (dt_name or DT, add_w or ADD_W)
    if key not in _CACHE:
        _CACHE[key] = _build(dt_name=dt_name, add_w=add_w)
    nc = _CACHE[key]
    in_maps = _host_prep(
        np.asarray(inputs["input1"], np.float32),
        np.asarray(inputs["input2"], np.float32),
        np.asarray(inputs["W1"], np.float32),
        np.asarray(inputs["b1"], np.float32),
        np.asarray(inputs["W2"], np.float32),
        np.asarray(inputs["Wo"], np.float32),
        dt_name=dt_name,
    )
    res = run_bass_kernel_spmd(
        nc, in_maps, list(range(NCORES)), trace=trace, **spmd_kwargs
    )
    out = _host_post(res.results, np.asarray(inputs["bo"], np.float32))
    return out, res


def kernel(**inputs) -> np.ndarray:
    out, _ = run(inputs, trace=False)
    return out


if __name__ == "__main__":
    rng = np.random.default_rng(0)
    ins = {
        "input1": rng.standard_normal((B, N1, D), dtype=np.float32),
        "input2": rng.standard_normal((B, N2, D), dtype=np.float32),
        "W1": rng.standard_normal((HID, D), dtype=np.float32) * 0.036,
        "b1": rng.standard_normal((HID,), dtype=np.float32) * 0.036,
        "W2": rng.standard_normal((HID, D), dtype=np.float32) * 0.036,
        "Wo": rng.standard_normal((OUT, HID), dtype=np.float32) * 0.031,
        "bo": rng.standard_normal((OUT,), dtype=np.float32) * 0.031,
    }
    out = kernel(**ins)
    print("kernel out", out.shape, out.dtype)


# revision 8
# speedup vs baseline: 1.1403x; 1.1088x over previous
"""Trainium2 Bass kernel for nn_BiClassifier (bilinear pairwise MLP).

Math (per batch b):
    in1 = input1 @ W1.T + b1            # [N1, HID]
    in2 = input2 @ W2.T                 # [N2, HID]
    h   = relu(in1[:,None,:] + in2[None,:,:])   # [N1, N2, HID]  (never materialized)
    out = h @ Wo.T + bo                 # [N1, N2, OUT]

Strategy: shard the 512 (b, n1) rows across 8 cores (64 rows each, one batch
per core pair). Weights are replicated. On each core the work is streamed
per hid-block hp (8 blocks of 128 on the partition axis):
  Phase A(hp) on PE: in1T[:, hp] [128, 64] and in2T[:, hp] [128, 128] from
      host-pre-transposed weights/inputs; in2T stays resident in PSUM for the
      ACT consumers and is copied to SBUF bf16 for the DVE consumers.
  Phase B(hp): 8 double-tiles h2 [128, 1024] = relu(in2T + in1T[:, n]) for
      group g's 4 rows of BOTH row-supers (one tensor_scalar / activation per
      row, FD=128 — per-partition scalar caps FD). Each h2 feeds two PE
      matmuls against the same wote [128, 16] stationary (Wo embedded at psum
      rows 2g:2g+2), accumulating [16, 512] PSUM banks (one per super) across
      all hp. Phase A(hp+1) is emitted after g0 so PE prefetches it into the
      producer-limited window.
Producers are split V:A by weighted round-robin (~71:29 balances DVE's
~165ns/instr against ACT's ~400ns/instr; both are dispatch-bound).
Host pre/post: transpose/shard inputs, unscramble output, add bo.
"""

import sys

import numpy as np

_REPO = "/opt/trn_rl_repo"
if _REPO not in sys.path:
    sys.path.insert(0, _REPO)

import concourse.bass as bass
import concourse.mybir as mybir
import concourse.tile as tile
from concourse import bacc
from concourse.bass_utils import run_bass_kernel_spmd

B, N1, N2, D, HID, OUT = 4, 128, 128, 768, 1024, 2
NCORES = 8
NR = 64            # (b, n1) rows per core
DB = D // 128      # 6 contraction blocks for the input projections
HP = HID // 128    # 8 hid blocks
NSUP = 2           # row supers per core (32 rows each -> one PSUM bank)
NG = 8             # row groups per super
GR = 4             # rows per group (group -> one [128, 512] h half-tile)

# Engine weights (V=Vector, A=Scalar/ACT) for h2-tile production.
ADD_W = (66, 34)
# Data dtype for weights/inputs/h tiles. PSUM accumulation and out stay fp32.
DT = "bfloat16"

_CACHE = {}


def _wrr(weights, n):
    """Weighted round-robin schedule of 'V'/'A' over n slots."""
    names = "VA"
    credits = [0.0, 0.0]
    total = float(sum(weights))
    out = []
    for _ in range(n):
        credits = [c + w for c, w in zip(credits, weights)]
        i = max(range(2), key=lambda k: credits[k])
        credits[i] -= total
        out.append(names[i])
    return out


def _build(dt_name=None, add_w=None):
    f32 = mybir.dt.float32
    dt = getattr(mybir.dt, dt_name or DT)
    sched = _wrr(add_w or ADD_W, HP * NG)
    # Bacc (not plain Bass): its finalize() runs the walrus legalization
    # passes (move_matmul_waits_to_ldweights, event semaphores, ...) without
    # which multi-wait instructions fail neuronxcc codegen.
    nc = bacc.Bacc(None, target_bir_lowering=False)

    w1 = nc.declare_dram_parameter("w1", [128, HP * DB * 128], dt, isOutput=False)
    w2 = nc.declare_dram_parameter("w2", [128, HP * DB * 128], dt, isOutput=False)
    wote = nc.declare_dram_parameter("wote", [128, HP * NG * 16], dt, isOutput=False)
    b1s = nc.declare_dram_parameter("b1s", [128, HP], f32, isOutput=False)
    x1 = nc.declare_dram_parameter("x1", [128, DB * NR], dt, isOutput=False)
    x2 = nc.declare_dram_parameter("x2", [128, DB * N2], dt, isOutput=False)
    out = nc.declare_dram_parameter("out", [16, NSUP * 512], f32, isOutput=True)

    with tile.TileContext(nc) as tc:
        with (
            tc.tile_pool(name="const", bufs=1) as cpool,
            tc.tile_pool(name="wpool", bufs=1) as wpool,
            tc.tile_pool(name="hpool", bufs=6) as hpool,
            tc.tile_pool(name="pa1", bufs=2, space=bass.MemorySpace.PSUM) as pa1,
            tc.tile_pool(name="pa2", bufs=2, space=bass.MemorySpace.PSUM) as pa2,
            tc.tile_pool(name="po", bufs=2, space=bass.MemorySpace.PSUM) as po,
        ):
            x1sb = cpool.tile([128, DB * NR], dt)
            x2sb = cpool.tile([128, DB * N2], dt)
            b1sb = cpool.tile([128, HP], f32)
            wotesb = cpool.tile([128, HP * NG * 16], dt)
            # in1t (fp32) feeds per-partition scalar reads (ACTIVATE bias /
            # tensor_scalar), which are fp32-only.
            in1t = cpool.tile([128, HP * NR], f32)
            in2t = cpool.tile([128, HP * N2], dt)
            outsb = cpool.tile([16, NSUP * 512], f32)

            # Per-hp weight tiles so phase A hp can start as soon as its
            # slice lands.
            w1sb = []
            w2sb = []
            for hp in range(HP):
                t1 = wpool.tile([128, DB * 128], dt, tag=f"w1_{hp}")
                t2 = wpool.tile([128, DB * 128], dt, tag=f"w2_{hp}")
                w1sb.append(t1)
                w2sb.append(t2)

            def _load_w(hp, eng=None):
                (eng or nc.sync).dma_start(
                    out=w1sb[hp][:], in_=w1[:, hp * DB * 128 : (hp + 1) * DB * 128]
                )
                (eng or nc.sync).dma_start(
                    out=w2sb[hp][:], in_=w2[:, hp * DB * 128 : (hp + 1) * DB * 128]
                )

            # DMA fill: the ramp is bounded by issue cost (~0.6us per
            # dma_start, serial per queue) plus ~3us transfer per 196KB tile,
            # so the hp0-critical tiles are split in half and issued in
            # parallel across the two HWDGE queues (sync + scalar).
            HB = DB // 2 * 128
            nc.sync.dma_start(out=w1sb[0][:, :HB], in_=w1[:, :HB])
            nc.scalar.dma_start(out=x1sb[:], in_=x1[:])
            nc.sync.dma_start(out=w1sb[0][:, HB:], in_=w1[:, HB : DB * 128])
            nc.scalar.dma_start(out=b1sb[:], in_=b1s[:])
            nc.sync.dma_start(out=x2sb[:], in_=x2[:])
            nc.scalar.dma_start(out=w2sb[0][:, :HB], in_=w2[:, :HB])
            nc.scalar.dma_start(
                out=w2sb[0][:, HB:], in_=w2[:, HB : DB * 128]
            )
            _load_w(1)
            nc.sync.dma_start(out=wotesb[:], in_=wote[:])
            for hp in range(2, HP):
                _load_w(hp)

            # Dummy activation up front: pulls the ~2.7us ACT table load into
            # the DMA fill window instead of the first real relu.
            warm = cpool.tile([128, 1], f32, name="warm")
            nc.vector.memset(warm[:], 0.0)
            nc.scalar.activation(
                warm[:], warm[:], mybir.ActivationFunctionType.Relu, bias=0.0,
                scale=1.0,
            )

            # Output accumulators: one [16, 512] bank per row-super, live for
            # the whole phase-B accumulation (all hp).
            pso = [po.tile([16, 512], f32, name=f"pso{s}") for s in range(NSUP)]

            def phase_a(hp):
                ps1 = pa1.tile([128, NR], f32, tag="ps1")
                for db in range(DB):
                    nc.tensor.matmul(
                        ps1[:],
                        w1sb[hp][:, db * 128 : (db + 1) * 128],
                        x1sb[:, db * NR : (db + 1) * NR],
                        start=(db == 0),
                        stop=(db == DB - 1),
                    )
                ps2 = pa2.tile([128, N2], f32, tag="ps2")
                for db in range(DB):
                    nc.tensor.matmul(
                        ps2[:],
                        w2sb[hp][:, db * 128 : (db + 1) * 128],
                        x2sb[:, db * N2 : (db + 1) * N2],
                        start=(db == 0),
                        stop=(db == DB - 1),
                    )
                return ps1, ps2

            def a_aux(hp, ps1, ps2):
                # in1t slice (fp32, += b1, Identity keeps negatives) and bf16
                # SBUF copy of in2T — both on ACT to keep the DVE queue clean.
                nc.scalar.activation(
                    in1t[:, hp * NR : (hp + 1) * NR],
                    ps1[:],
                    mybir.ActivationFunctionType.Identity,
                    bias=b1sb[:, hp : hp + 1],
                    scale=1.0,
                )
                nc.scalar.copy(in2t[:, hp * N2 : (hp + 1) * N2], ps2[:])

            cur = phase_a(0)
            a_aux(0, *cur)

            for hp in range(HP):
                ps2 = cur[1]
                src = in2t[:, hp * N2 : (hp + 1) * N2]
                for g in range(NG):
                    h2 = hpool.tile([128, NSUP * GR * N2], dt, tag="h")
                    which = sched[hp * NG + g]
                    for sup in range(NSUP):
                        for j in range(GR):
                            row = sup * (NG * GR) + g * GR + j
                            col = in1t[:, hp * NR + row : hp * NR + row + 1]
                            dst = h2[:, (sup * GR + j) * N2 : (sup * GR + j + 1) * N2]
                            if which == "A":
                                # src from SBUF (not PSUM): ACT's PSUM reads
                                # contend with PE's PSUM writes, and reading
                                # SBUF releases the ps2 bank right after the
                                # cast instead of after ~50 A-quarters.
                                nc.scalar.activation(
                                    dst,
                                    src,
                                    mybir.ActivationFunctionType.Relu,
                                    bias=col,
                                    scale=1.0,
                                )
                            else:
                                nc.vector.tensor_scalar(
                                    dst,
                                    src,
                                    col,
                                    0.0,
                                    mybir.AluOpType.add,
                                    mybir.AluOpType.max,
                                )
                    wslice = wotesb[
                        :, hp * NG * 16 + g * 16 : hp * NG * 16 + (g + 1) * 16
                    ]
                    for sup in range(NSUP):
                        nc.tensor.matmul(
                            pso[sup][:],
                            wslice,
                            h2[:, sup * GR * N2 : (sup + 1) * GR * N2],
                            start=(hp == 0 and g == 0),
                            stop=(hp == HP - 1 and g == NG - 1),
                        )
                    if g == 0 and hp + 1 < HP:
                        # Prefetch next hp's projections into the PE bubble;
                        # producers for hp still have ~7 tiles queued.
                        cur = phase_a(hp + 1)
                        a_aux(hp + 1, *cur)

            # Evacuate on both engines in parallel to shorten the tail.
            nc.vector.tensor_copy(outsb[:, 0:512], pso[0][:])
            nc.scalar.copy(outsb[:, 512:1024], pso[1][:])
            nc.sync.dma_start(out=out[:], in_=outsb[:])

    nc.finalize()
    return nc


def _np_dt(dt_name):
    if dt_name == "bfloat16":
        import ml_dtypes

        return ml_dtypes.bfloat16
    return np.float32


def _host_prep(input1, input2, W1, b1, W2, Wo, dt_name=None):
    f32 = np.float32
    dt = _np_dt(dt_name or DT)
    c = np.ascontiguousarray

    # w[p, hp, db, j] = W[hp*128+j, db*128+p]
    w1sb = c(W1.reshape(HP, 128, DB, 128).transpose(3, 0, 2, 1).reshape(128, -1), dt)
    w2sb = c(W2.reshape(HP, 128, DB, 128).transpose(3, 0, 2, 1).reshape(128, -1), dt)

    # wote[p, hp, s, 2s+o] = Wo[o, hp*128+p]
    wo_hpo = Wo.T.reshape(HP, 128, OUT)  # [hp, p, o]
    wote = np.zeros((128, HP, NG, 16), f32)
    for s in range(NG):
        wote[:, :, s, 2 * s : 2 * s + 2] = wo_hpo.transpose(1, 0, 2)
    wote = c(wote.reshape(128, -1), dt)

    b1sb = c(b1.reshape(HP, 128).T, f32)

    in_maps = []
    for core in range(NCORES):
        b, half = core // 2, core % 2
        n0 = half * NR
        x1sb = c(
            input1[b, n0 : n0 + NR].reshape(NR, DB, 128).transpose(2, 1, 0).reshape(128, -1),
            dt,
        )
        x2sb = c(
            input2[b].reshape(N2, DB, 128).transpose(2, 1, 0).reshape(128, -1), dt
        )
        in_maps.append(
            {"w1": w1sb, "w2": w2sb, "wote": wote, "b1s": b1sb, "x1": x1sb, "x2": x2sb}
        )
    return in_maps


def _host_post(results, bo):
    out_full = np.empty((B, N1, N2, OUT), np.float32)
    for core in range(NCORES):
        b, half = core // 2, core % 2
        co = np.asarray(results[core]["out"], np.float32)
        co = co.reshape(NG, OUT, NSUP, GR, N2)  # [s, o, sup, j, m]
        arr = co.transpose(2, 0, 3, 4, 1).reshape(NR, N2, OUT)  # [sup,s,j] -> rows
        out_full[b, half * NR : (half + 1) * NR] = arr
    out_full += np.asarray(bo, np.float32)
    return out_full


def run(inputs, trace=False, dt_name=None, add_w=None, **spmd_kwargs):
    """Run on hardware; returns (output, BassKernelResults)."""
    key = (dt_name or DT, add_w or ADD_W)
    if key not in _CACHE:
        _CACHE[key] = _build(dt_name=dt_name, add_w=add_w)
    nc = _CACHE[key]
    in_maps = _host_prep(
        np.asarray(inputs["input1"], np.float32),
        np.asarray(inputs["input2"], np.float32),
        np.asarray(inputs["W1"], np.float32),
        np.asarray(inputs["b1"], np.float32),
        np.asarray(inputs["W2"], np.float32),
        np.asarray(inputs["Wo"], np.float32),
        dt_name=dt_name,
    )
    res = run_bass_kernel_spmd(
        nc, in_maps, list(range(NCORES)), trace=trace, **spmd_kwargs
    )
    out = _host_post(res.results, np.asarray(inputs["bo"], np.float32))
    return out, res


def kernel(**inputs) -> np.ndarray:
    out, _ = run(inputs, trace=False)
    return out


if __name__ == "__main__":
    rng = np.random.default_rng(0)
    ins = {
        "input1": rng.standard_normal((B, N1, D), dtype=np.float32),
        "input2": rng.standard_normal((B, N2, D), dtype=np.float32),
        "W1": rng.standard_normal((HID, D), dtype=np.float32) * 0.036,
        "b1": rng.standard_normal((HID,), dtype=np.float32) * 0.036,
        "W2": rng.standard_normal((HID, D), dtype=np.float32) * 0.036,
        "Wo": rng.standard_normal((OUT, HID), dtype=np.float32) * 0.031,
        "bo": rng.standard_normal((OUT,), dtype=np.float32) * 0.031,
    }
    out = kernel(**ins)
    print("kernel out", out.shape, out.dtype)
